# revision 17
# baseline (speedup 1.0000x reference)
"""Causal self-attention TRN2 kernel.

Full inputs in, full output out. Sharding: core c = 4*b + g runs batch b
(of 2) and head-group g (4 of 16 heads). Host pre-transposes each shard so
every SBUF layout is natural for the PE, and casts to bf16 (error budget
2e-2; bf16 end-to-end lands ~1e-2 absmax):

  xT  [1024, 2048] = x[b].T                      (bf16)
  wqT/wkT/wvT [1024, 256] = w[rows of group].T   (bf16; wq pre-scaled 1/8)
  woT [256, 1024] = wo[:, cols of group].T       (bf16)

Per core (all matmul inputs bf16, PSUM accum fp32):
  qT,kT [256,2048] = (wT).T-chunks @ xT      (contraction over D)
  v     [2048,256] = xT-chunks.T @ wvT       (natural layout, k on partition)
  ST[k,q] tiles    = kT-chunk.T @ qT-chunk   (K=64; 2 heads packed via PE
                                              row-tiles at partitions 0/64)
  E = exp(ST) on ScalarE straight from PSUM (softmax max-subtraction is
      skipped: scores are ~N(0,1), max < ~7, exp never overflows);
      causal mask only on the diagonal 128x128 block of each boundary tile
      (0/1 multiply on DVE, bf16 2x mode)
  AV: out.T[65,q] += [v_h | ones].T @ E      (ones column makes row 64 the
                                              softmax denominator for free)
  normalize: outer-product broadcast of 1/rowsum (K=1 matmul) and one
      tensor_tensor multiply at PSUM eviction
  y[t,:] partial = attnoutT-chunks.T @ woT   (bf16 out; host sums the 4
                                              group partials in fp32)

Projection window tj and attention window qj=tj are interleaved in emission
order so ScalarE exp (the attention bottleneck) overlaps PE projection work.
"""

from contextlib import ExitStack

import numpy as np

from concourse import bacc, bass, mybir, tile
from concourse.bass_utils import run_bass_kernel_spmd
from concourse.masks import make_upper_triangular

B, T, D = 2, 2048, 1024
H, DH = 16, 64
N_CORES = 8
HG = 4                # tensor-parallel groups
HPG = H // HG         # heads per group = 4
CL = HPG * DH         # local channels = 256
KC = D // 128         # contraction chunks over D = 8
TQ = T // 512         # 512-wide T windows = 4
F32 = mybir.dt.float32
BF16 = mybir.dt.bfloat16
PAIRED = True
ET_BUFS = 8


class Ctx:
    pass


def emit_consts(ctx, tc, g, wqT, wkT, wvT, woT):
    nc = tc.nc
    persist = ctx.enter_context(tc.tile_pool(name="persist", bufs=1))
    g.xt_pool = ctx.enter_context(tc.tile_pool(name="xt", bufs=3))
    g.et_pool = ctx.enter_context(tc.tile_pool(name="et", bufs=ET_BUFS))
    g.ysb_pool = ctx.enter_context(tc.tile_pool(name="ysb", bufs=4))
    g.rc_pool = ctx.enter_context(tc.tile_pool(name="rc", bufs=3))
    # One PSUM pool, 8 banks: tag "ps512" 4 slots (qk/st/y), "psB" 4 (v/av/rb)
    g.pp = ctx.enter_context(tc.tile_pool(name="pp", bufs=4, space="PSUM"))

    g.mask01 = persist.tile([128, 128], BF16, tag="mask01", name="mask01")
    make_upper_triangular(nc, g.mask01[:, :], val=1.0, diag=True)

    g.ones_col = persist.tile([1, 64], BF16, tag="ones", name="ones")
    nc.vector.memset(g.ones_col[:, :], 1.0)

    # merged weight tiles: chunk kc of wX lives at cols CL*kc (one DMA each)
    g.wq_all = persist.tile([128, KC * CL], BF16, tag="wq_all", name="wq_all")
    g.wk_all = persist.tile([128, KC * CL], BF16, tag="wk_all", name="wk_all")
    g.wv_all = persist.tile([128, KC * CL], BF16, tag="wv_all", name="wv_all")
    g.wo_all = persist.tile([128, 2 * D], BF16, tag="wo_all", name="wo_all")
    g.wq_sb = [g.wq_all[:, CL * i:CL * i + CL] for i in range(KC)]
    g.wk_sb = [g.wk_all[:, CL * i:CL * i + CL] for i in range(KC)]
    g.wv_sb = [g.wv_all[:, CL * i:CL * i + CL] for i in range(KC)]
    g.wo_sb = [g.wo_all[:, D * i:D * i + D] for i in range(2)]
    # weight DMAs are issued inside emit_proj(0) (after the first x window,
    # interleaved per projection) so the PE can start ~1us into the kernel

    g.qT_sb = [persist.tile([128, T], BF16, tag=f"qT{i}", name=f"qT{i}") for i in range(2)]
    g.kT_sb = [persist.tile([128, T], BF16, tag=f"kT{i}", name=f"kT{i}") for i in range(2)]
    g.aT_sb = [persist.tile([128, T], BF16, tag=f"aT{i}", name=f"aT{i}") for i in range(2)]

    # v natural layout, one tile per 128-row k-chunk, head-strided cols of 65
    # (col 65h+64 is the ones column for the softmax denominator trick)
    g.v_sb = [persist.tile([128, HPG * 65], BF16, tag=f"v{i}", name=f"v{i}")
              for i in range(T // 128)]
    for i in range(T // 128):
        ones_cols = g.v_sb[i].rearrange("p (h c) -> p h c", c=65)[:, :, 64:65]
        nc.vector.memset(ones_cols, 1.0)


def emit_xt_dma(tc, g, xT, tj):
    nc = tc.nc
    ts = 512 * tj
    xt_all = g.xt_pool.tile([128, KC * 512], BF16, tag="xt", name="xt")
    # four DMAs (2 kc-chunks each): finer dependency pacing so the first
    # matmul only waits on its own chunk, few dispatches
    for quarter in range(4):
        nc.sync.dma_start(
            out=xt_all.rearrange("p (kc t) -> p kc t", t=512)[:, 2 * quarter:2 * quarter + 2],
            in_=xT.rearrange("(kc p) t -> p kc t", p=128)[:, 2 * quarter:2 * quarter + 2,
                                                          ts:ts + 512],
        )
    return xt_all


def emit_qk_group(tc, g, xt_all, tj, which, m, borrow=False):
    """One q/k projection PSUM group: 8 accumulating matmuls + DVE evict."""
    nc = tc.nc
    ts = 512 * tj
    xt = [xt_all[:, 512 * kc:512 * kc + 512] for kc in range(KC)]
    w_sb, dst = (g.wq_sb, g.qT_sb) if which == "q" else (g.wk_sb, g.kT_sb)
    if borrow:
        # startup: the av slots are idle until the first AV matmul -- borrow
        # them so the four startup q/k PSUM groups double-buffer
        ps = g.pp.tile([128, 512], F32, tag="av", bufs=2, name="psqk")
    else:
        ps = g.pp.tile([128, 512], F32, tag="pj", bufs=1, name="psqk")
    for kc in range(KC):
        nc.tensor.matmul(
            out=ps[:, :],
            lhsT=(w_sb[kc][:, 128 * m:128 * m + 128]),
            rhs=(xt[kc][:, :]),
            start=(kc == 0),
            stop=(kc == KC - 1),
        )
    nc.vector.tensor_copy(dst[m][:, ts:ts + 512], ps[:, :])


def emit_v_group(tc, g, xt_all, tj, tc4):
    """One v projection PSUM group (natural [t, ch] layout) + DVE evict."""
    nc = tc.nc
    xt = [xt_all[:, 512 * kc:512 * kc + 512] for kc in range(KC)]
    tg = 4 * tj + tc4
    ps = g.pp.tile([128, CL], F32, tag="pj", bufs=1, name="psv")
    for kc in range(KC):
        nc.tensor.matmul(
            out=ps[:, :],
            lhsT=(xt[kc][:, 128 * tc4:128 * tc4 + 128]),
            rhs=(g.wv_sb[kc][:, :]),
            start=(kc == 0),
            stop=(kc == KC - 1),
        )
    nc.vector.tensor_copy(
        g.v_sb[tg].rearrange("p (h c) -> p h c", c=65)[:, :, 0:64],
        ps.rearrange("p (h c) -> p h c", c=64)[:, :, :],
    )


def emit_weight_dma(tc, g, which, wT):
    nc = tc.nc
    if which == "wo":
        nc.scalar.dma_start(
            out=g.wo_all.rearrange("p (cc d) -> p cc d", d=D),
            in_=wT.rearrange("(cc p) d -> p cc d", p=128),
        )
        return
    w_all = {"wq": g.wq_all, "wk": g.wk_all, "wv": g.wv_all}[which]
    for half in range(2):
        nc.scalar.dma_start(
            out=w_all.rearrange("p (kc c) -> p kc c", c=CL)[:, 4 * half:4 * half + 4],
            in_=wT.rearrange("(kc p) c -> p kc c", p=128)[:, 4 * half:4 * half + 4],
        )


def emit_attn(tc, g, y, qj, phase="all", stash=None):
    """Scores+exp+mask and AV for window qj.

    Normalization is split per head-pair: hp0's rb/aT-multiply interleaves
    into hp1's pair loop (so hp1's av-ring allocation can proceed); hp1's is
    returned and deferred into the next S-stage where independent projection
    groups hide it."""
    nc = tc.nc
    qs = 512 * qj
    nk = 4 * qj + 4  # k-chunks 0..nk-1 reach this window

    def geom(ki):
        if ki < 4 * qj:
            return 512, 0
        w = 512 - 128 * (ki - 4 * qj)
        return w, 512 - w

    def emit_scores(hp, pi):
        if PAIRED:
            ki0, ki1 = 2 * pi, 2 * pi + 1
        else:
            ki0 = ki1 = pi
        w0, qoff0 = geom(ki0)
        w1, qoff1 = geom(ki1)
        ets = []
        for hh in range(2):  # packed PE row-tiles (base partition 0/64)
            po = 64 * hh
            if PAIRED:
                st = g.pp.tile([128, 1024], F32, tag="st", bufs=2, name="st")
                plan = ((ki0, w0, qoff0, 0), (ki1, w1, qoff1, w0))
            else:
                st = g.pp.tile([128, 512], F32, tag="st", bufs=3, name="st")
                plan = ((ki0, w0, qoff0, 0),)
            for (ki, w, qoff, co) in plan:
                nc.tensor.matmul(
                    out=st[:, co:co + w],
                    lhsT=(g.kT_sb[hp][po:po + 64, 128 * ki:128 * ki + 128]),
                    rhs=(g.qT_sb[hp][po:po + 64, qs + qoff:qs + 512]),
                    start=True,
                    stop=True,
                )
            wid = w0 + w1 if PAIRED else w0
            et = g.et_pool.tile([128, 1024 if PAIRED else 512], BF16, tag="et",
                                name="et")
            nc.scalar.activation(
                out=et[:, :wid],
                in_=st[:, :wid],
                func=mybir.ActivationFunctionType.Exp,
            )
            # diagonal 128x128 blocks need the causal mask (gpsimd: SBUF-only
            # op, keeps DVE free for PSUM evictions)
            if ki0 >= 4 * qj:
                nc.gpsimd.tensor_mul(et[:, 0:128], et[:, 0:128], g.mask01[:, :])
            if PAIRED and ki1 >= 4 * qj:
                nc.gpsimd.tensor_mul(et[:, w0:w0 + 128], et[:, w0:w0 + 128],
                                     g.mask01[:, :])
            ets.append(et)
        return ets

    def emit_av(hp, pi, av, ets):
        if PAIRED:
            ki0, ki1 = 2 * pi, 2 * pi + 1
        else:
            ki0 = ki1 = pi
        w0, qoff0 = geom(ki0)
        w1, qoff1 = geom(ki1)
        for hh in range(2):
            h = 2 * hp + hh
            nc.tensor.matmul(
                out=av[hh][:, qoff0:512],
                lhsT=(g.v_sb[ki0][:, 65 * h:65 * h + 65]),
                rhs=(ets[hh][:, :w0]),
                start=(ki0 == 0),
                stop=(not PAIRED and ki0 == nk - 1),
            )
            if PAIRED:
                nc.tensor.matmul(
                    out=av[hh][:, qoff1:512],
                    lhsT=(g.v_sb[ki1][:, 65 * h:65 * h + 65]),
                    rhs=(ets[hh][:, w0:w0 + w1]),
                    start=False,
                    stop=(ki1 == nk - 1),
                )

    def emit_recips(av):
        recips = []
        for hh in range(2):
            recip_r = g.rc_pool.tile([1, 512], BF16, tag="recip", name="recip")
            with nc.allow_low_precision(reason="bf16 softmax denominator"):
                nc.vector.reciprocal(recip_r[:, :], av[hh][64:65, :])
            recips.append(recip_r)
        return recips

    npair = nk // 2 if PAIRED else nk
    if phase == "scores":
        for hp in range(2):
            for pi in range(npair):
                stash[(hp, pi)] = emit_scores(hp, pi)
        return None
    if phase == "av":
        st0 = None
        for hp in range(2):
            av = [g.pp.tile([65, 512], F32, tag="av", bufs=2, name="av")
                  for _ in range(2)]
            if hp == 1:  # free hp0's av slots before hp1 writes its own
                for hh in range(2):
                    emit_normB_half(tc, g, qj, 0, hh, st0)
            for pi in range(npair):
                emit_av(hp, pi, av, stash[(hp, pi)])
            if hp == 0:
                st0 = (av, emit_recips(av))
            else:
                return (av, emit_recips(av))
    # steady state: software-pipeline AV D tiles behind scores so the PE
    # never waits on the exp->mask chain
    D = 1 if PAIRED else 2
    st0 = None
    for hp in range(2):
        av = [g.pp.tile([65, 512], F32, tag="av", bufs=2, name="av")
              for _ in range(2)]
        fillers = ([lambda hh=hh: emit_normB_half(tc, g, qj, 0, hh, st0)
                    for hh in range(2)] if hp == 1 else [])
        pend = []
        for pi in range(npair):
            pend.append((pi, emit_scores(hp, pi)))
            if fillers:
                fillers.pop(0)()
            if len(pend) > D:
                pj_, ets_ = pend.pop(0)
                emit_av(hp, pj_, av, ets_)
        for f in fillers:
            f()
        for (pj_, ets_) in pend:
            emit_av(hp, pj_, av, ets_)
        if hp == 0:
            st0 = (av, emit_recips(av))
        else:
            return (av, emit_recips(av))


def emit_normB_half(tc, g, qj, hp, hh, state):
    """gpsimd partition-broadcast of 1/rowsum + aT multiply for one (hp, hh).

    partition_broadcast runs on the otherwise-idle Pool engine (SBUF-only),
    replacing the old ones-column outer-product matmul + PSUM evict."""
    nc = tc.nc
    qs = 512 * qj
    av, recips = state
    po = 64 * hh
    rb_sb = g.rc_pool.tile([64, 512], BF16, tag="rb_sb", name="rb_sb")
    nc.gpsimd.partition_broadcast(rb_sb[:, :], recips[hh][:, :])
    nc.vector.tensor_mul(
        g.aT_sb[hp][po:po + 64, qs:qs + 512], av[hh][0:64, :], rb_sb[:, :]
    )


def emit_py_group(tc, g, y, pw, tc4, dj, ysb_map):
    """One outproj PSUM group + evict (Act/DVE alternating); DMA on dj=1."""
    nc = tc.nc
    tg = 4 * pw + tc4
    if dj == 0:
        ysb_map[tc4] = g.ysb_pool.tile([128, D], BF16, tag="ysb", name="ysb")
    ysb = ysb_map[tc4]
    py = g.pp.tile([128, 512], F32, tag="py", bufs=1, name="py")
    for cc in range(2):
        nc.tensor.matmul(
            out=py[:, :],
            lhsT=(g.aT_sb[cc][:, 128 * tg:128 * tg + 128]),
            rhs=(g.wo_sb[cc][:, 512 * dj:512 * dj + 512]),
            start=(cc == 0),
            stop=(cc == 1),
        )
    if (tc4 + dj) % 2 == 0:
        nc.scalar.copy(ysb[:, 512 * dj:512 * dj + 512], py[:, :])
    else:
        nc.vector.tensor_copy(ysb[:, 512 * dj:512 * dj + 512], py[:, :])
    if dj == 1:
        nc.sync.dma_start(out=y[128 * tg:128 * tg + 128, :], in_=ysb[:, :])


def emit_S(tc, g, y, xT, tj, pend):
    """Interleaved stage: hp1-normB + outproj of the previous window braided
    with the q/k/v projections of window tj, so every PSUM-group eviction
    hides behind an independent matmul group on the PE."""
    pw, state = pend if pend is not None else (None, None)
    ysb_map = {}
    if tj is not None:
        xt_all = emit_xt_dma(tc, g, xT, tj)
        emit_qk_group(tc, g, xt_all, tj, "q", 0)
        if state is not None:
            emit_normB_half(tc, g, pw, 1, 0, state)
        emit_qk_group(tc, g, xt_all, tj, "q", 1)
        if state is not None:
            emit_normB_half(tc, g, pw, 1, 1, state)
        emit_qk_group(tc, g, xt_all, tj, "k", 0)
        emit_qk_group(tc, g, xt_all, tj, "k", 1)
        py_jobs = ([(tc4, dj) for tc4 in range(4) for dj in range(2)]
                   if state is not None else [])
        for tc4 in range(4):
            emit_v_group(tc, g, xt_all, tj, tc4)
            for (t, d) in py_jobs[2 * tc4:2 * tc4 + 2]:
                emit_py_group(tc, g, y, pw, t, d, ysb_map)
    else:  # tail: no next window, just normB + outproj
        emit_normB_half(tc, g, pw, 1, 0, state)
        emit_normB_half(tc, g, pw, 1, 1, state)
        for tc4 in range(4):
            for dj in range(2):
                emit_py_group(tc, g, y, pw, tc4, dj, ysb_map)
    return


def attn_kernel(ctx, tc, y, xT, wqT, wkT, wvT, woT, n_reps=1):
    g = Ctx()
    emit_consts(ctx, tc, g, wqT, wkT, wvT, woT)
    pend = None  # (window, hp1 norm state) awaiting normB + outproj
    for rep in range(n_reps):
        for w in range(TQ):
            if rep == 0 and w == 0:
                xt_all = emit_xt_dma(tc, g, xT, 0)
                emit_weight_dma(tc, g, "wq", wqT)
                emit_qk_group(tc, g, xt_all, 0, "q", 0, borrow=True)
                emit_qk_group(tc, g, xt_all, 0, "q", 1, borrow=True)
                emit_weight_dma(tc, g, "wk", wkT)
                emit_qk_group(tc, g, xt_all, 0, "k", 0, borrow=True)
                emit_qk_group(tc, g, xt_all, 0, "k", 1, borrow=True)
                stash = {}
                emit_attn(tc, g, y, 0, phase="scores", stash=stash)
                emit_weight_dma(tc, g, "wv", wvT)
                for tc4 in range(4):
                    emit_v_group(tc, g, xt_all, 0, tc4)
                emit_weight_dma(tc, g, "wo", woT)
                state = emit_attn(tc, g, y, 0, phase="av", stash=stash)
            else:
                emit_S(tc, g, y, xT, w, pend)
                state = emit_attn(tc, g, y, w)
            pend = (w, state)
    emit_S(tc, g, y, None, None, pend)
    return


_PROGRAMS = {}


def get_program(n_reps=1):
    key = (n_reps, PAIRED, ET_BUFS)
    if key not in _PROGRAMS:
        nc = bacc.Bacc("TRN2", target_bir_lowering=False, debug=False,
                       num_devices=N_CORES)
        xT = nc.dram_tensor("xT", [D, T], BF16, kind="ExternalInput").ap()
        wqT = nc.dram_tensor("wqT", [D, CL], BF16, kind="ExternalInput").ap()
        wkT = nc.dram_tensor("wkT", [D, CL], BF16, kind="ExternalInput").ap()
        wvT = nc.dram_tensor("wvT", [D, CL], BF16, kind="ExternalInput").ap()
        woT = nc.dram_tensor("woT", [CL, D], BF16, kind="ExternalInput").ap()
        y = nc.dram_tensor("y", [T, D], BF16, kind="ExternalOutput").ap()
        with tile.TileContext(nc) as tc:
            with ExitStack() as ctx:
                attn_kernel(ctx, tc, y, xT, wqT, wkT, wvT, woT, n_reps=n_reps)
        nc.compile()
        _PROGRAMS[key] = nc
    return _PROGRAMS[key]


def make_in_maps(x, wq, wk, wv, wo):
    import ml_dtypes
    bf16 = ml_dtypes.bfloat16
    x = np.asarray(x, np.float32)
    wq, wk, wv, wo = (np.asarray(a, np.float32) for a in (wq, wk, wv, wo))
    scale = np.float32(DH ** -0.5)
    in_maps = []
    for c in range(N_CORES):
        b, g = divmod(c, HG)
        rows = slice(g * CL, (g + 1) * CL)
        in_maps.append({
            "xT": np.ascontiguousarray(x[b].T).astype(bf16),
            # score scale 1/sqrt(DH) folded into wq on the host
            "wqT": (np.ascontiguousarray(wq[rows].T) * scale).astype(bf16),
            "wkT": np.ascontiguousarray(wk[rows].T).astype(bf16),
            "wvT": np.ascontiguousarray(wv[rows].T).astype(bf16),
            "woT": np.ascontiguousarray(wo[:, rows].T).astype(bf16),
        })
    return in_maps


def gather(results):
    y = np.zeros((B, T, D), np.float32)
    for c in range(N_CORES):
        y[c // HG] += np.asarray(results[c]["y"], np.float32)
    return y


def kernel(x, wq, wk, wv, wo):
    nc = get_program()
    in_maps = make_in_maps(x, wq, wk, wv, wo)
    res = run_bass_kernel_spmd(nc, in_maps, list(range(N_CORES)))
    return gather(res.results)


# revision 19
# speedup vs baseline: 1.3095x; 1.3095x over previous
"""Causal self-attention TRN2 kernel.

Full inputs in, full output out. Sharding: core c = 4*b + g runs batch b
(of 2) and head-group g (4 of 16 heads). Host pre-transposes each shard so
every SBUF layout is natural for the PE, and casts to bf16 (error budget
2e-2; bf16 end-to-end lands ~1e-2 absmax):

  xT  [1024, 2048] = x[b].T                      (bf16)
  wqT/wkT/wvT [1024, 256] = w[rows of group].T   (bf16; wq pre-scaled 1/8)
  woT [256, 1024] = wo[:, cols of group].T       (bf16)

Per core (all matmul inputs bf16, PSUM accum fp32):
  qT,kT [256,2048] = (wT).T-chunks @ xT      (contraction over D)
  v     [2048,256] = xT-chunks.T @ wvT       (natural layout, k on partition)
  ST[k,q] tiles    = kT-chunk.T @ qT-chunk   (K=64; 2 heads packed via PE
                                              row-tiles at partitions 0/64)
  E = exp(ST) on ScalarE straight from PSUM (softmax max-subtraction is
      skipped: scores are ~N(0,1), max < ~7, exp never overflows);
      causal mask only on the diagonal 128x128 block of each boundary tile
      (0/1 multiply on DVE, bf16 2x mode)
  AV: out.T[65,q] += [v_h | ones].T @ E      (ones column makes row 64 the
                                              softmax denominator for free)
  normalize: outer-product broadcast of 1/rowsum (K=1 matmul) and one
      tensor_tensor multiply at PSUM eviction
  y[t,:] partial = attnoutT-chunks.T @ woT   (bf16 out; host sums the 4
                                              group partials in fp32)

Projection window tj and attention window qj=tj are interleaved in emission
order so ScalarE exp (the attention bottleneck) overlaps PE projection work.
"""

from contextlib import ExitStack

import numpy as np

from concourse import bacc, bass, mybir, tile
from concourse.bass_utils import run_bass_kernel_spmd
from concourse.masks import make_upper_triangular

B, T, D = 2, 2048, 1024
H, DH = 16, 64
N_CORES = 8
HG = 4                # tensor-parallel groups
HPG = H // HG         # heads per group = 4
CL = HPG * DH         # local channels = 256
KC = D // 128         # contraction chunks over D = 8
TQ = T // 512         # 512-wide T windows = 4
F32 = mybir.dt.float32
BF16 = mybir.dt.bfloat16
PAIRED = True
ET_BUFS = 8


class Ctx:
    pass


def emit_consts(ctx, tc, g, wqT, wkT, wvT, woT):
    nc = tc.nc
    persist = ctx.enter_context(tc.tile_pool(name="persist", bufs=1))
    g.xt_pool = ctx.enter_context(tc.tile_pool(name="xt", bufs=3))
    g.et_pool = ctx.enter_context(tc.tile_pool(name="et", bufs=ET_BUFS))
    g.ysb_pool = ctx.enter_context(tc.tile_pool(name="ysb", bufs=4))
    g.rc_pool = ctx.enter_context(tc.tile_pool(name="rc", bufs=3))
    # One PSUM pool, 8 banks: tag "ps512" 4 slots (qk/st/y), "psB" 4 (v/av/rb)
    g.pp = ctx.enter_context(tc.tile_pool(name="pp", bufs=4, space="PSUM"))

    g.mask01 = persist.tile([128, 128], BF16, tag="mask01", name="mask01")
    make_upper_triangular(nc, g.mask01[:, :], val=1.0, diag=True)

    g.ones_col = persist.tile([1, 64], BF16, tag="ones", name="ones")
    nc.vector.memset(g.ones_col[:, :], 1.0)

    # merged weight tiles: chunk kc of wX lives at cols CL*kc (one DMA each)
    g.wq_all = persist.tile([128, KC * CL], BF16, tag="wq_all", name="wq_all")
    g.wk_all = persist.tile([128, KC * CL], BF16, tag="wk_all", name="wk_all")
    g.wv_all = persist.tile([128, KC * CL], BF16, tag="wv_all", name="wv_all")
    g.wo_all = persist.tile([128, 2 * D], BF16, tag="wo_all", name="wo_all")
    g.wq_sb = [g.wq_all[:, CL * i:CL * i + CL] for i in range(KC)]
    g.wk_sb = [g.wk_all[:, CL * i:CL * i + CL] for i in range(KC)]
    g.wv_sb = [g.wv_all[:, CL * i:CL * i + CL] for i in range(KC)]
    g.wo_sb = [g.wo_all[:, D * i:D * i + D] for i in range(2)]
    # weight DMAs are issued inside emit_proj(0) (after the first x window,
    # interleaved per projection) so the PE can start ~1us into the kernel

    g.qT_sb = [persist.tile([128, T], BF16, tag=f"qT{i}", name=f"qT{i}") for i in range(2)]
    g.kT_sb = [persist.tile([128, T], BF16, tag=f"kT{i}", name=f"kT{i}") for i in range(2)]
    g.aT_sb = [persist.tile([128, T], BF16, tag=f"aT{i}", name=f"aT{i}") for i in range(2)]

    # v natural layout, one tile per 128-row k-chunk, head-strided cols of 65
    # (col 65h+64 is the ones column for the softmax denominator trick)
    g.v_sb = [persist.tile([128, HPG * 65], BF16, tag=f"v{i}", name=f"v{i}")
              for i in range(T // 128)]
    for i in range(T // 128):
        ones_cols = g.v_sb[i].rearrange("p (h c) -> p h c", c=65)[:, :, 64:65]
        nc.vector.memset(ones_cols, 1.0)


def emit_xt_dma(tc, g, xT, tj):
    nc = tc.nc
    ts = 512 * tj
    xt_all = g.xt_pool.tile([128, KC * 512], BF16, tag="xt", name="xt")
    # four DMAs (2 kc-chunks each): finer dependency pacing so the first
    # matmul only waits on its own chunk, few dispatches
    for quarter in range(4):
        nc.sync.dma_start(
            out=xt_all.rearrange("p (kc t) -> p kc t", t=512)[:, 2 * quarter:2 * quarter + 2],
            in_=xT.rearrange("(kc p) t -> p kc t", p=128)[:, 2 * quarter:2 * quarter + 2,
                                                          ts:ts + 512],
        )
    return xt_all


def emit_qk_group(tc, g, xt_all, tj, which, m, borrow=False):
    """One q/k projection PSUM group: 8 accumulating matmuls + DVE evict."""
    nc = tc.nc
    ts = 512 * tj
    xt = [xt_all[:, 512 * kc:512 * kc + 512] for kc in range(KC)]
    w_sb, dst = (g.wq_sb, g.qT_sb) if which == "q" else (g.wk_sb, g.kT_sb)
    if borrow:
        # startup: the av slots are idle until the first AV matmul -- borrow
        # them so the four startup q/k PSUM groups double-buffer
        ps = g.pp.tile([128, 512], F32, tag="av", bufs=2, name="psqk")
    else:
        ps = g.pp.tile([128, 512], F32, tag="pj", bufs=1, name="psqk")
    for kc in range(KC):
        nc.tensor.matmul(
            out=ps[:, :],
            lhsT=(w_sb[kc][:, 128 * m:128 * m + 128]),
            rhs=(xt[kc][:, :]),
            start=(kc == 0),
            stop=(kc == KC - 1),
        )
    nc.vector.tensor_copy(dst[m][:, ts:ts + 512], ps[:, :])


def emit_v_group(tc, g, xt_all, tj, tc4):
    """One v projection PSUM group (natural [t, ch] layout) + DVE evict."""
    nc = tc.nc
    xt = [xt_all[:, 512 * kc:512 * kc + 512] for kc in range(KC)]
    tg = 4 * tj + tc4
    ps = g.pp.tile([128, CL], F32, tag="pj", bufs=1, name="psv")
    for kc in range(KC):
        nc.tensor.matmul(
            out=ps[:, :],
            lhsT=(xt[kc][:, 128 * tc4:128 * tc4 + 128]),
            rhs=(g.wv_sb[kc][:, :]),
            start=(kc == 0),
            stop=(kc == KC - 1),
        )
    nc.vector.tensor_copy(
        g.v_sb[tg].rearrange("p (h c) -> p h c", c=65)[:, :, 0:64],
        ps.rearrange("p (h c) -> p h c", c=64)[:, :, :],
    )


def emit_weight_dma(tc, g, which, wT):
    nc = tc.nc
    if which == "wo":
        nc.scalar.dma_start(
            out=g.wo_all.rearrange("p (cc d) -> p cc d", d=D),
            in_=wT.rearrange("(cc p) d -> p cc d", p=128),
        )
        return
    w_all = {"wq": g.wq_all, "wk": g.wk_all, "wv": g.wv_all}[which]
    for half in range(2):
        nc.scalar.dma_start(
            out=w_all.rearrange("p (kc c) -> p kc c", c=CL)[:, 4 * half:4 * half + 4],
            in_=wT.rearrange("(kc p) c -> p kc c", p=128)[:, 4 * half:4 * half + 4],
        )


def emit_attn(tc, g, y, qj, phase="all", stash=None):
    """Scores+exp+mask and AV for window qj.

    Normalization is split per head-pair: hp0's rb/aT-multiply interleaves
    into hp1's pair loop (so hp1's av-ring allocation can proceed); hp1's is
    returned and deferred into the next S-stage where independent projection
    groups hide it."""
    nc = tc.nc
    qs = 512 * qj
    nk = 4 * qj + 4  # k-chunks 0..nk-1 reach this window

    def geom(ki):
        if ki < 4 * qj:
            return 512, 0
        w = 512 - 128 * (ki - 4 * qj)
        return w, 512 - w

    def emit_scores(hp, pi):
        if PAIRED:
            ki0, ki1 = 2 * pi, 2 * pi + 1
        else:
            ki0 = ki1 = pi
        w0, qoff0 = geom(ki0)
        w1, qoff1 = geom(ki1)
        ets = []
        for hh in range(2):  # packed PE row-tiles (base partition 0/64)
            po = 64 * hh
            if PAIRED:
                st = g.pp.tile([128, 1024], F32, tag="st", bufs=2, name="st")
                plan = ((ki0, w0, qoff0, 0), (ki1, w1, qoff1, w0))
            else:
                st = g.pp.tile([128, 512], F32, tag="st", bufs=3, name="st")
                plan = ((ki0, w0, qoff0, 0),)
            for (ki, w, qoff, co) in plan:
                nc.tensor.matmul(
                    out=st[:, co:co + w],
                    lhsT=(g.kT_sb[hp][po:po + 64, 128 * ki:128 * ki + 128]),
                    rhs=(g.qT_sb[hp][po:po + 64, qs + qoff:qs + 512]),
                    start=True,
                    stop=True,
                )
            wid = w0 + w1 if PAIRED else w0
            et = g.et_pool.tile([128, 1024 if PAIRED else 512], BF16, tag="et",
                                name="et")
            nc.scalar.activation(
                out=et[:, :wid],
                in_=st[:, :wid],
                func=mybir.ActivationFunctionType.Exp,
            )
            # diagonal 128x128 blocks need the causal mask (DVE 2x mode:
            # all-SBUF bf16 operands)
            if ki0 >= 4 * qj:
                nc.vector.tensor_mul(et[:, 0:128], et[:, 0:128], g.mask01[:, :])
            if PAIRED and ki1 >= 4 * qj:
                nc.vector.tensor_mul(et[:, w0:w0 + 128], et[:, w0:w0 + 128],
                                     g.mask01[:, :])
            ets.append(et)
        return ets

    def emit_av(hp, pi, av, ets):
        if PAIRED:
            ki0, ki1 = 2 * pi, 2 * pi + 1
        else:
            ki0 = ki1 = pi
        w0, qoff0 = geom(ki0)
        w1, qoff1 = geom(ki1)
        for hh in range(2):
            h = 2 * hp + hh
            nc.tensor.matmul(
                out=av[hh][:, qoff0:512],
                lhsT=(g.v_sb[ki0][:, 65 * h:65 * h + 65]),
                rhs=(ets[hh][:, :w0]),
                start=(ki0 == 0),
                stop=(not PAIRED and ki0 == nk - 1),
            )
            if PAIRED:
                nc.tensor.matmul(
                    out=av[hh][:, qoff1:512],
                    lhsT=(g.v_sb[ki1][:, 65 * h:65 * h + 65]),
                    rhs=(ets[hh][:, w0:w0 + w1]),
                    start=False,
                    stop=(ki1 == nk - 1),
                )

    def emit_recips(av):
        recips = []
        for hh in range(2):
            recip_r = g.rc_pool.tile([1, 512], BF16, tag="recip", name="recip")
            with nc.allow_low_precision(reason="bf16 softmax denominator"):
                nc.vector.reciprocal(recip_r[:, :], av[hh][64:65, :])
            recips.append(recip_r)
        return recips

    npair = nk // 2 if PAIRED else nk
    if phase == "scores":
        for hp in range(2):
            for pi in range(npair):
                stash[(hp, pi)] = emit_scores(hp, pi)
        return None
    if phase == "av":
        st0 = None
        for hp in range(2):
            av = [g.pp.tile([65, 512], F32, tag="av", bufs=2, name="av")
                  for _ in range(2)]
            if hp == 1:  # free hp0's av slots before hp1 writes its own
                for hh in range(2):
                    emit_normB_half(tc, g, qj, 0, hh, st0)
            for pi in range(npair):
                emit_av(hp, pi, av, stash[(hp, pi)])
            if hp == 0:
                st0 = (av, emit_recips(av))
            else:
                return (av, emit_recips(av))
    # steady state: software-pipeline AV D tiles behind scores so the PE
    # never waits on the exp->mask chain
    D = 1 if PAIRED else 2
    st0 = None
    for hp in range(2):
        av = [g.pp.tile([65, 512], F32, tag="av", bufs=2, name="av")
              for _ in range(2)]
        fillers = ([lambda hh=hh: emit_normB_half(tc, g, qj, 0, hh, st0)
                    for hh in range(2)] if hp == 1 else [])
        pend = []
        for pi in range(npair):
            pend.append((pi, emit_scores(hp, pi)))
            if fillers:
                fillers.pop(0)()
            if len(pend) > D:
                pj_, ets_ = pend.pop(0)
                emit_av(hp, pj_, av, ets_)
        for f in fillers:
            f()
        for (pj_, ets_) in pend:
            emit_av(hp, pj_, av, ets_)
        if hp == 0:
            st0 = (av, emit_recips(av))
        else:
            return (av, emit_recips(av))


def emit_normB_half(tc, g, qj, hp, hh, state):
    """rb broadcast matmul + rb_sb evict + aT multiply for one (hp, hh)."""
    nc = tc.nc
    qs = 512 * qj
    av, recips = state
    po = 64 * hh
    rb = g.pp.tile([64, 512], F32, tag="py", bufs=1, name="rb")
    nc.tensor.matmul(
        out=rb[:, :],
        lhsT=(g.ones_col[:, :]),
        rhs=(recips[hh][:, :]),
        start=True,
        stop=True,
    )
    rb_sb = g.rc_pool.tile([64, 512], BF16, tag="rb_sb", name="rb_sb")
    nc.vector.tensor_copy(rb_sb[:, :], rb[:, :])
    nc.vector.tensor_mul(
        g.aT_sb[hp][po:po + 64, qs:qs + 512], av[hh][0:64, :], rb_sb[:, :]
    )


def emit_py_group(tc, g, y, pw, tc4, dj, ysb_map):
    """One outproj PSUM group + evict (Act/DVE alternating); DMA on dj=1."""
    nc = tc.nc
    tg = 4 * pw + tc4
    if dj == 0:
        ysb_map[tc4] = g.ysb_pool.tile([128, D], BF16, tag="ysb", name="ysb")
    ysb = ysb_map[tc4]
    py = g.pp.tile([128, 512], F32, tag="py", bufs=1, name="py")
    for cc in range(2):
        nc.tensor.matmul(
            out=py[:, :],
            lhsT=(g.aT_sb[cc][:, 128 * tg:128 * tg + 128]),
            rhs=(g.wo_sb[cc][:, 512 * dj:512 * dj + 512]),
            start=(cc == 0),
            stop=(cc == 1),
        )
    if (tc4 + dj) % 2 == 0:
        nc.scalar.copy(ysb[:, 512 * dj:512 * dj + 512], py[:, :])
    else:
        nc.vector.tensor_copy(ysb[:, 512 * dj:512 * dj + 512], py[:, :])
    if dj == 1:
        nc.sync.dma_start(out=y[128 * tg:128 * tg + 128, :], in_=ysb[:, :])


def emit_S(tc, g, y, xT, tj, pend):
    """Interleaved stage: hp1-normB + outproj of the previous window braided
    with the q/k/v projections of window tj, so every PSUM-group eviction
    hides behind an independent matmul group on the PE."""
    pw, state = pend if pend is not None else (None, None)
    ysb_map = {}
    if tj is not None:
        xt_all = emit_xt_dma(tc, g, xT, tj)
        emit_qk_group(tc, g, xt_all, tj, "q", 0)
        if state is not None:
            emit_normB_half(tc, g, pw, 1, 0, state)
        emit_qk_group(tc, g, xt_all, tj, "q", 1)
        if state is not None:
            emit_normB_half(tc, g, pw, 1, 1, state)
        emit_qk_group(tc, g, xt_all, tj, "k", 0)
        emit_qk_group(tc, g, xt_all, tj, "k", 1)
        py_jobs = ([(tc4, dj) for tc4 in range(4) for dj in range(2)]
                   if state is not None else [])
        for tc4 in range(4):
            emit_v_group(tc, g, xt_all, tj, tc4)
            for (t, d) in py_jobs[2 * tc4:2 * tc4 + 2]:
                emit_py_group(tc, g, y, pw, t, d, ysb_map)
    else:  # tail: no next window, just normB + outproj
        emit_normB_half(tc, g, pw, 1, 0, state)
        emit_normB_half(tc, g, pw, 1, 1, state)
        for tc4 in range(4):
            for dj in range(2):
                emit_py_group(tc, g, y, pw, tc4, dj, ysb_map)
    return


def attn_kernel(ctx, tc, y, xT, wqT, wkT, wvT, woT, n_reps=1):
    g = Ctx()
    emit_consts(ctx, tc, g, wqT, wkT, wvT, woT)
    pend = None  # (window, hp1 norm state) awaiting normB + outproj
    for rep in range(n_reps):
        for w in range(TQ):
            if rep == 0 and w == 0:
                xt_all = emit_xt_dma(tc, g, xT, 0)
                emit_weight_dma(tc, g, "wq", wqT)
                emit_qk_group(tc, g, xt_all, 0, "q", 0, borrow=True)
                emit_qk_group(tc, g, xt_all, 0, "q", 1, borrow=True)
                emit_weight_dma(tc, g, "wk", wkT)
                emit_qk_group(tc, g, xt_all, 0, "k", 0, borrow=True)
                emit_qk_group(tc, g, xt_all, 0, "k", 1, borrow=True)
                stash = {}
                emit_attn(tc, g, y, 0, phase="scores", stash=stash)
                emit_weight_dma(tc, g, "wv", wvT)
                for tc4 in range(4):
                    emit_v_group(tc, g, xt_all, 0, tc4)
                emit_weight_dma(tc, g, "wo", woT)
                state = emit_attn(tc, g, y, 0, phase="av", stash=stash)
            else:
                emit_S(tc, g, y, xT, w, pend)
                state = emit_attn(tc, g, y, w)
            pend = (w, state)
    emit_S(tc, g, y, None, None, pend)
    return


_PROGRAMS = {}


def get_program(n_reps=1):
    key = (n_reps, PAIRED, ET_BUFS)
    if key not in _PROGRAMS:
        nc = bacc.Bacc("TRN2", target_bir_lowering=False, debug=False,
                       num_devices=N_CORES)
        xT = nc.dram_tensor("xT", [D, T], BF16, kind="ExternalInput").ap()
        wqT = nc.dram_tensor("wqT", [D, CL], BF16, kind="ExternalInput").ap()
        wkT = nc.dram_tensor("wkT", [D, CL], BF16, kind="ExternalInput").ap()
        wvT = nc.dram_tensor("wvT", [D, CL], BF16, kind="ExternalInput").ap()
        woT = nc.dram_tensor("woT", [CL, D], BF16, kind="ExternalInput").ap()
        y = nc.dram_tensor("y", [T, D], BF16, kind="ExternalOutput").ap()
        with tile.TileContext(nc) as tc:
            with ExitStack() as ctx:
                attn_kernel(ctx, tc, y, xT, wqT, wkT, wvT, woT, n_reps=n_reps)
        nc.compile()
        _PROGRAMS[key] = nc
    return _PROGRAMS[key]


def make_in_maps(x, wq, wk, wv, wo):
    import ml_dtypes
    bf16 = ml_dtypes.bfloat16
    x = np.asarray(x, np.float32)
    wq, wk, wv, wo = (np.asarray(a, np.float32) for a in (wq, wk, wv, wo))
    scale = np.float32(DH ** -0.5)
    in_maps = []
    for c in range(N_CORES):
        b, g = divmod(c, HG)
        rows = slice(g * CL, (g + 1) * CL)
        in_maps.append({
            "xT": np.ascontiguousarray(x[b].T).astype(bf16),
            # score scale 1/sqrt(DH) folded into wq on the host
            "wqT": (np.ascontiguousarray(wq[rows].T) * scale).astype(bf16),
            "wkT": np.ascontiguousarray(wk[rows].T).astype(bf16),
            "wvT": np.ascontiguousarray(wv[rows].T).astype(bf16),
            "woT": np.ascontiguousarray(wo[:, rows].T).astype(bf16),
        })
    return in_maps


def gather(results):
    y = np.zeros((B, T, D), np.float32)
    for c in range(N_CORES):
        y[c // HG] += np.asarray(results[c]["y"], np.float32)
    return y


def kernel(x, wq, wk, wv, wo):
    nc = get_program()
    in_maps = make_in_maps(x, wq, wk, wv, wo)
    res = run_bass_kernel_spmd(nc, in_maps, list(range(N_CORES)))
    return gather(res.results)
